# revision 19
# baseline (speedup 1.0000x reference)
"""Trainium2 Bass kernel for 3D neighborhood attention — v3.

Same decomposition as v2 (see kernel.py docstring) but instruction mix
tuned to measured TRN2 DVE behavior:
  - two-tensor ops only via TENSOR_TENSOR (runs 2x with packed fp16;
    SCALAR_TENSOR_TENSOR measures 1x regardless of dtype),
  - per (dj,di) group: one coarse 4-dim-view product instruction and a
    4-instruction pairwise d-reduction tree (the rel-pos bias block,
    host-expanded to the full logits layout, enters as a tree leaf).
"""

import numpy as np

import concourse.bass as bass
import concourse.tile as tile
from concourse import bacc, mybir
from concourse.ap import AP
from concourse.bass_utils import run_bass_kernel_spmd

NH = 8
HD = 6
DIM = NH * HD
KS = 3
SCALE = HD**-0.5
SHIFT = 2.0
H = W = T = 40
N_CORES = 8
SLAB = H // N_CORES
NLC = 3
LCH = 14
P_OUT = W * NLC              # 120
P_K = (W + 2) * NLC          # 126
ILH = SLAB * LCH             # 70
QF = SLAB * LCH * DIM        # 3360
KI = SLAB + 2                # 7
KL = LCH + 2                 # 16
KF = KI * KL * DIM           # 5376
LF = 27 * ILH * NH           # 15120
DJF = 9 * ILH * NH           # 5040 per-dj logits block
OF = 3 * ILH * NH            # 1680

MULT = mybir.AluOpType.mult
ADD = mybir.AluOpType.add
SUB = mybir.AluOpType.subtract

_prog_cache = {}


def _ap(base, offset, dims):
    return AP(base.tensor, base.offset + offset,
              [list(base.ap[0])] + [list(d) for d in dims])


def _build_program():
    fp16 = mybir.dt.float16
    fp32 = mybir.dt.float32
    nc = bacc.Bacc("TRN2", target_bir_lowering=False, debug=False,
                   num_devices=N_CORES)
    qd = nc.dram_tensor("qd", [P_OUT, QF], fp16, kind="ExternalInput").ap()
    kd = nc.dram_tensor("kd", [P_K, KF], fp16, kind="ExternalInput").ap()
    rd = nc.dram_tensor("rd", [P_OUT, 27 * NH], fp16,
                        kind="ExternalInput").ap()
    od = nc.dram_tensor("od", [P_OUT, OF], fp16, kind="ExternalOutput").ap()

    def tt(out, in0, in1, op, acc=None):
        eng = nc.vector
        return eng.add_instruction(mybir.InstTensorTensor(
            name=nc.get_next_instruction_name(),
            op=op, acc=acc,
            ins=[eng.lower_ap(in0), eng.lower_ap(in1)],
            outs=[eng.lower_ap(out)],
        ))

    with tile.TileContext(nc) as tc:
        with (
            tc.tile_pool(name="io", bufs=1) as io,
            tc.tile_pool(name="work", bufs=1) as wk,
        ):
            rpb = io.tile([P_OUT, 27 * NH], fp16)
            nc.sync.dma_start(rpb[:], rd[:])
            q = io.tile([P_OUT, QF], fp16)
            kt = [io.tile([P_OUT, KF], fp16, name=f"kt{dj}")
                  for dj in range(3)]
            KC = KF // 8
            QC = QF // 4
            def kchunk(c):
                eng = nc.sync if c % 2 == 0 else nc.scalar
                eng.dma_start(kt[0][:, c * KC:(c + 1) * KC],
                              kd[0:P_OUT, c * KC:(c + 1) * KC])
            def qchunk(c):
                eng = nc.sync if c % 2 == 0 else nc.scalar
                eng.dma_start(q[:, c * QC:(c + 1) * QC],
                              qd[:, c * QC:(c + 1) * QC])
            qchunk(0)
            kchunk(1)
            kchunk(0)
            qchunk(1)
            kchunk(2)
            kchunk(3)
            qchunk(2)
            qchunk(3)
            for c in (4, 5, 6, 7):
                kchunk(c)
            for dj in range(1, 3):
                for c in range(8):
                    eng = nc.sync if (c + dj) % 2 == 0 else nc.scalar
                    eng.dma_start(
                        kt[dj][:, c * KC:(c + 1) * KC],
                        kd[3 * dj:3 * dj + P_OUT, c * KC:(c + 1) * KC])

            L = wk.tile([P_OUT, LF], fp16)   # (dj, di, dl, il, h)
            E = wk.tile([P_OUT, LF], fp16)
            Sdj = []
            Tdj = []
            dls = []
            vs = []

            qv4 = _ap(q[:], 0, [(0, 3), (672, 5), (48, 14), (1, 48)])
            for dj in range(3):
                for di in range(3):
                    # products for all (dl, d, h) of this (dj, di) in one
                    # TENSOR_TENSOR (2x with packed fp16); P4 = (dl, il, dh)
                    P4 = wk.tile([P_OUT, 3 * ILH * DIM], fp16)
                    if dj == 0 and di == 0:
                        # split by i so compute starts before the full
                        # q/k0 startup DMA completes
                        for i0, ic in ((0, 1), (1, 1), (2, 3)):
                            kvh = _ap(kt[0][:], i0 * (KL * DIM),
                                      [(48, 3), (768, ic), (48, 14),
                                       (1, 48)])
                            qvh = _ap(q[:], i0 * 672,
                                      [(0, 3), (672, ic), (48, 14),
                                       (1, 48)])
                            pvh = _ap(P4[:], i0 * 672,
                                      [(3360, 3), (672, ic), (48, 14),
                                       (1, 48)])
                            tt(pvh, qvh, kvh, MULT)
                    else:
                        kv4 = _ap(kt[dj][:], di * (KL * DIM),
                                  [(48, 3), (768, 5), (48, 14), (1, 48)])
                        pv4 = _ap(P4[:], 0, [(3360, 3), (672, 5), (48, 14),
                                             (1, 48)])
                        tt(pv4, qv4, kv4, MULT)
                    # d-reduction tree: 6 -> 3 -> (+rpb) -> 1
                    A = wk.tile([P_OUT, 3 * ILH * 3 * NH], fp16)
                    av = _ap(A[:], 0, [(1680, 3), (24, 70), (8, 3), (1, 8)])
                    p0 = _ap(P4[:], 0, [(3360, 3), (48, 70), (8, 3), (1, 8)])
                    p1 = _ap(P4[:], 24, [(3360, 3), (48, 70), (8, 3), (1, 8)])
                    tt(av, p0, p1, ADD)
                    A2 = wk.tile([P_OUT, 3 * ILH * NH], fp16)
                    a0 = _ap(A[:], 0, [(1680, 3), (24, 70), (1, 8)])
                    a1 = _ap(A[:], 8, [(1680, 3), (24, 70), (1, 8)])
                    tt(A2[:], a0, a1, ADD)
                    A3 = wk.tile([P_OUT, 3 * ILH * NH], fp16)
                    a2 = _ap(A[:], 16, [(1680, 3), (24, 70), (1, 8)])
                    rv = _ap(rpb[:], di * 72 + dj * 24,
                             [(8, 3), (0, 70), (1, 8)])
                    tt(A3[:], a2, rv, ADD)
                    lv = _ap(L[:], dj * DJF + di * 1680,
                             [(560, 3), (8, 70), (1, 8)])
                    tt(lv, A2[:], A3[:], ADD)
                    o0 = dj * DJF + di * 1680
                    nc.scalar.activation(E[:, o0:o0 + 1680],
                                         L[:, o0:o0 + 1680],
                                         mybir.ActivationFunctionType.Exp)
                e0 = E[:, dj * DJF:dj * DJF + 1680]
                e1 = E[:, dj * DJF + 1680:dj * DJF + 3360]
                e2 = E[:, dj * DJF + 3360:dj * DJF + 5040]
                s1 = wk.tile([P_OUT, 1680], fp16, name=f"s1_{dj}")
                nc.vector.tensor_add(s1[:], e0, e1)
                sd = wk.tile([P_OUT, 1680], fp16, name=f"sd{dj}")
                nc.vector.tensor_add(sd[:], s1[:], e2)
                Sdj.append(sd)
                t1 = wk.tile([P_OUT, 560], fp16, name=f"t1_{dj}")
                nc.vector.tensor_add(t1[:], sd[:, 0:560], sd[:, 560:1120])
                td = wk.tile([P_OUT, 560], fp16, name=f"td{dj}")
                nc.vector.tensor_add(td[:], t1[:], sd[:, 1120:1680])
                Tdj.append(td)
                dd = wk.tile([P_OUT, 560], fp16, name=f"dd{dj}")
                nc.vector.tensor_sub(dd[:], sd[:, 1120:1680], sd[:, 0:560])
                dls.append(dd)
                vv = wk.tile([P_OUT, 1680], fp16, name=f"vv{dj}")
                nc.vector.tensor_sub(vv[:], e2, e0)
                vs.append(vv)
                if dj == 1:
                    d1 = wk.tile([P_OUT, 560], fp16)
                    nc.vector.tensor_add(d1[:], Tdj[0][:], Tdj[1][:])
                    nl1 = wk.tile([P_OUT, 560], fp16)
                    nc.vector.tensor_add(nl1[:], dls[0][:], dls[1][:])
                    v1 = wk.tile([P_OUT, 1680], fp16)
                    nc.vector.tensor_add(v1[:], vs[0][:], vs[1][:])

            # endgame ordered by readiness: denom/recip first, then the
            # cheap numerators (nj, nl) multiply + stream out while the
            # longer ni chain still runs
            denom = wk.tile([P_OUT, 560], fp32)
            nc.vector.tensor_add(denom[:], d1[:], Tdj[2][:])
            r32 = wk.tile([P_OUT, 560], fp32)
            nc.vector.reciprocal_approx_fast(r32[:], denom[:])
            r16 = wk.tile([P_OUT, 560], fp16)
            nc.scalar.copy(r16[:], r32[:])
            out = wk.tile([P_OUT, OF], fp16)

            nj = wk.tile([P_OUT, 560], fp16)
            nc.vector.tensor_sub(nj[:], Tdj[2][:], Tdj[0][:])
            nc.vector.tensor_mul(out[:, 560:1120], nj[:], r16[:])
            nc.scalar.dma_start(od[:, 560:1120], out[:, 560:1120])

            nl = wk.tile([P_OUT, 560], fp16)
            nc.vector.tensor_add(nl[:], nl1[:], dls[2][:])
            nc.vector.tensor_mul(out[:, 1120:1680], nl[:], r16[:])
            nc.sync.dma_start(od[:, 1120:1680], out[:, 1120:1680])

            v2 = wk.tile([P_OUT, 1680], fp16)
            nc.vector.tensor_add(v2[:], v1[:], vs[2][:])
            ni1 = wk.tile([P_OUT, 560], fp16)
            nc.vector.tensor_add(ni1[:], v2[:, 0:560], v2[:, 560:1120])
            ni = wk.tile([P_OUT, 560], fp16)
            nc.vector.tensor_add(ni[:], ni1[:], v2[:, 1120:1680])
            nc.vector.tensor_mul(out[:, 0:560], ni[:], r16[:])
            nc.scalar.dma_start(od[:, 0:560], out[:, 0:560])

    nc.compile()
    return nc


_PERM = np.array([(c % 8) * 6 + c // 8 for c in range(DIM)])


def _host_prep(q, k, rpb):
    f16 = np.float16
    q0 = (np.asarray(q, np.float32)[0] * SCALE).astype(f16)[..., _PERM]
    k0 = np.asarray(k, np.float32)[0].astype(f16)[..., _PERM]
    kp = np.pad(k0, ((1, 1), (1, 1), (1, 3), (0, 0)))     # [42,42,44,48]
    qp = np.pad(q0, ((0, 0), (0, 0), (0, 2), (0, 0)))     # [40,40,42,48]

    rpb16 = (np.asarray(rpb, np.float32) - SHIFT).astype(f16)  # [8,3,3,3]
    rpb_t = np.ascontiguousarray(
        rpb16.transpose(1, 2, 3, 0)).reshape(27 * NH)     # (di,dj,dl,h)
    rpbx = np.broadcast_to(rpb_t, (P_OUT, 27 * NH)).copy()

    in_maps = []
    for core in range(N_CORES):
        i0 = core * SLAB
        qs = qp[i0:i0 + SLAB]
        qt = qs.reshape(SLAB, W, NLC, LCH, DIM).transpose(1, 2, 0, 3, 4)
        qt = np.ascontiguousarray(qt).reshape(P_OUT, QF)
        ks = kp[i0:i0 + KI]
        idx_l = (np.arange(NLC)[:, None] * LCH + np.arange(KL)[None, :])
        kk = ks[:, :, idx_l]
        kk = kk.transpose(1, 2, 0, 3, 4)
        kk = np.ascontiguousarray(kk).reshape(P_K, KF)
        in_maps.append({"qd": qt, "kd": kk, "rd": rpbx})
    return in_maps


def _assemble(results):
    full = np.zeros((H, W, T, NH, 3), np.float32)
    for core in range(N_CORES):
        i0 = core * SLAB
        o = results[core]["od"].astype(np.float32)
        o = o.reshape(W, NLC, 3, SLAB, LCH, NH)
        for lc in range(NLC):
            nl_ = LCH if lc < 2 else T - 2 * LCH
            full[i0:i0 + SLAB, :, lc * LCH:lc * LCH + nl_] = (
                o[:, lc, :, :, :nl_].transpose(2, 0, 3, 4, 1))
    out = full.reshape(H, W, T, NH * 3).transpose(3, 0, 1, 2)[None]
    return np.ascontiguousarray(out)


def _run(q, k, rpb, **spmd_kwargs):
    if "prog" not in _prog_cache:
        _prog_cache["prog"] = _build_program()
    nc = _prog_cache["prog"]
    in_maps = _host_prep(q, k, rpb)
    res = run_bass_kernel_spmd(nc, in_maps, list(range(N_CORES)),
                               **spmd_kwargs)
    return _assemble(res.results), res


def kernel(q, k, rpb):
    out, _ = _run(q, k, rpb)
    return out


# revision 20
# speedup vs baseline: 1.0027x; 1.0027x over previous
"""Trainium2 Bass kernel for 3D neighborhood attention — v3.

Same decomposition as v2 (see kernel.py docstring) but instruction mix
tuned to measured TRN2 DVE behavior:
  - two-tensor ops only via TENSOR_TENSOR (runs 2x with packed fp16;
    SCALAR_TENSOR_TENSOR measures 1x regardless of dtype),
  - per (dj,di) group: one coarse 4-dim-view product instruction and a
    4-instruction pairwise d-reduction tree (the rel-pos bias block,
    host-expanded to the full logits layout, enters as a tree leaf).
"""

import numpy as np

import concourse.bass as bass
import concourse.tile as tile
from concourse import bacc, mybir
from concourse.ap import AP
from concourse.bass_utils import run_bass_kernel_spmd

NH = 8
HD = 6
DIM = NH * HD
KS = 3
SCALE = HD**-0.5
SHIFT = 2.0
H = W = T = 40
N_CORES = 8
SLAB = H // N_CORES
NLC = 3
LCH = 14
P_OUT = W * NLC              # 120
P_K = (W + 2) * NLC          # 126
ILH = SLAB * LCH             # 70
QF = SLAB * LCH * DIM        # 3360
KI = SLAB + 2                # 7
KL = LCH + 2                 # 16
KF = KI * KL * DIM           # 5376
LF = 27 * ILH * NH           # 15120
DJF = 9 * ILH * NH           # 5040 per-dj logits block
OF = 3 * ILH * NH            # 1680

MULT = mybir.AluOpType.mult
ADD = mybir.AluOpType.add
SUB = mybir.AluOpType.subtract

_prog_cache = {}


def _ap(base, offset, dims):
    return AP(base.tensor, base.offset + offset,
              [list(base.ap[0])] + [list(d) for d in dims])


def _build_program():
    fp16 = mybir.dt.float16
    fp32 = mybir.dt.float32
    nc = bacc.Bacc("TRN2", target_bir_lowering=False, debug=False,
                   num_devices=N_CORES)
    qd = nc.dram_tensor("qd", [P_OUT, QF], fp16, kind="ExternalInput").ap()
    kd = nc.dram_tensor("kd", [P_K, KF], fp16, kind="ExternalInput").ap()
    rd = nc.dram_tensor("rd", [P_OUT, 27 * NH], fp16,
                        kind="ExternalInput").ap()
    od = nc.dram_tensor("od", [P_OUT, OF], fp16, kind="ExternalOutput").ap()

    def tt(out, in0, in1, op, acc=None):
        eng = nc.vector
        return eng.add_instruction(mybir.InstTensorTensor(
            name=nc.get_next_instruction_name(),
            op=op, acc=acc,
            ins=[eng.lower_ap(in0), eng.lower_ap(in1)],
            outs=[eng.lower_ap(out)],
        ))

    with tile.TileContext(nc) as tc:
        with (
            tc.tile_pool(name="io", bufs=1) as io,
            tc.tile_pool(name="work", bufs=1) as wk,
        ):
            rpb = io.tile([P_OUT, 27 * NH], fp16)
            nc.sync.dma_start(rpb[:], rd[:])
            q = io.tile([P_OUT, QF], fp16)
            kt = [io.tile([P_OUT, KF], fp16, name=f"kt{dj}")
                  for dj in range(3)]
            KC = KF // 8
            QC = QF // 4
            def kchunk(c):
                eng = nc.sync if c % 2 == 0 else nc.scalar
                eng.dma_start(kt[0][:, c * KC:(c + 1) * KC],
                              kd[0:P_OUT, c * KC:(c + 1) * KC])
            def qchunk(c):
                eng = nc.sync if c % 2 == 0 else nc.scalar
                eng.dma_start(q[:, c * QC:(c + 1) * QC],
                              qd[:, c * QC:(c + 1) * QC])
            for c in (0, 1):
                qchunk(c)
            for c in (0, 1, 2, 3):
                kchunk(c)
            for c in (2, 3):
                qchunk(c)
            for c in (4, 5, 6, 7):
                kchunk(c)
            for dj in range(1, 3):
                for c in range(8):
                    eng = nc.sync if (c + dj) % 2 == 0 else nc.scalar
                    eng.dma_start(
                        kt[dj][:, c * KC:(c + 1) * KC],
                        kd[3 * dj:3 * dj + P_OUT, c * KC:(c + 1) * KC])

            L = wk.tile([P_OUT, LF], fp16)   # (dj, di, dl, il, h)
            E = wk.tile([P_OUT, LF], fp16)
            Sdj = []
            Tdj = []
            dls = []
            vs = []

            qv4 = _ap(q[:], 0, [(0, 3), (672, 5), (48, 14), (1, 48)])
            for dj in range(3):
                for di in range(3):
                    # products for all (dl, d, h) of this (dj, di) in one
                    # TENSOR_TENSOR (2x with packed fp16); P4 = (dl, il, dh)
                    P4 = wk.tile([P_OUT, 3 * ILH * DIM], fp16)
                    if dj == 0 and di == 0:
                        # split by i so compute starts before the full
                        # q/k0 startup DMA completes
                        for i0, ic in ((0, 2), (2, 3)):
                            kvh = _ap(kt[0][:], i0 * (KL * DIM),
                                      [(48, 3), (768, ic), (48, 14),
                                       (1, 48)])
                            qvh = _ap(q[:], i0 * 672,
                                      [(0, 3), (672, ic), (48, 14),
                                       (1, 48)])
                            pvh = _ap(P4[:], i0 * 672,
                                      [(3360, 3), (672, ic), (48, 14),
                                       (1, 48)])
                            tt(pvh, qvh, kvh, MULT)
                    else:
                        kv4 = _ap(kt[dj][:], di * (KL * DIM),
                                  [(48, 3), (768, 5), (48, 14), (1, 48)])
                        pv4 = _ap(P4[:], 0, [(3360, 3), (672, 5), (48, 14),
                                             (1, 48)])
                        tt(pv4, qv4, kv4, MULT)
                    # d-reduction tree: 6 -> 3 -> (+rpb) -> 1
                    A = wk.tile([P_OUT, 3 * ILH * 3 * NH], fp16)
                    av = _ap(A[:], 0, [(1680, 3), (24, 70), (8, 3), (1, 8)])
                    p0 = _ap(P4[:], 0, [(3360, 3), (48, 70), (8, 3), (1, 8)])
                    p1 = _ap(P4[:], 24, [(3360, 3), (48, 70), (8, 3), (1, 8)])
                    tt(av, p0, p1, ADD)
                    A2 = wk.tile([P_OUT, 3 * ILH * NH], fp16)
                    a0 = _ap(A[:], 0, [(1680, 3), (24, 70), (1, 8)])
                    a1 = _ap(A[:], 8, [(1680, 3), (24, 70), (1, 8)])
                    tt(A2[:], a0, a1, ADD)
                    A3 = wk.tile([P_OUT, 3 * ILH * NH], fp16)
                    a2 = _ap(A[:], 16, [(1680, 3), (24, 70), (1, 8)])
                    rv = _ap(rpb[:], di * 72 + dj * 24,
                             [(8, 3), (0, 70), (1, 8)])
                    tt(A3[:], a2, rv, ADD)
                    lv = _ap(L[:], dj * DJF + di * 1680,
                             [(560, 3), (8, 70), (1, 8)])
                    tt(lv, A2[:], A3[:], ADD)
                    o0 = dj * DJF + di * 1680
                    nc.scalar.activation(E[:, o0:o0 + 1680],
                                         L[:, o0:o0 + 1680],
                                         mybir.ActivationFunctionType.Exp)
                e0 = E[:, dj * DJF:dj * DJF + 1680]
                e1 = E[:, dj * DJF + 1680:dj * DJF + 3360]
                e2 = E[:, dj * DJF + 3360:dj * DJF + 5040]
                s1 = wk.tile([P_OUT, 1680], fp16, name=f"s1_{dj}")
                nc.vector.tensor_add(s1[:], e0, e1)
                sd = wk.tile([P_OUT, 1680], fp16, name=f"sd{dj}")
                nc.vector.tensor_add(sd[:], s1[:], e2)
                Sdj.append(sd)
                t1 = wk.tile([P_OUT, 560], fp16, name=f"t1_{dj}")
                nc.vector.tensor_add(t1[:], sd[:, 0:560], sd[:, 560:1120])
                td = wk.tile([P_OUT, 560], fp16, name=f"td{dj}")
                nc.vector.tensor_add(td[:], t1[:], sd[:, 1120:1680])
                Tdj.append(td)
                dd = wk.tile([P_OUT, 560], fp16, name=f"dd{dj}")
                nc.vector.tensor_sub(dd[:], sd[:, 1120:1680], sd[:, 0:560])
                dls.append(dd)
                vv = wk.tile([P_OUT, 1680], fp16, name=f"vv{dj}")
                nc.vector.tensor_sub(vv[:], e2, e0)
                vs.append(vv)
                if dj == 1:
                    d1 = wk.tile([P_OUT, 560], fp16)
                    nc.vector.tensor_add(d1[:], Tdj[0][:], Tdj[1][:])
                    nl1 = wk.tile([P_OUT, 560], fp16)
                    nc.vector.tensor_add(nl1[:], dls[0][:], dls[1][:])
                    v1 = wk.tile([P_OUT, 1680], fp16)
                    nc.vector.tensor_add(v1[:], vs[0][:], vs[1][:])

            # endgame ordered by readiness: denom/recip first, then the
            # cheap numerators (nj, nl) multiply + stream out while the
            # longer ni chain still runs
            denom = wk.tile([P_OUT, 560], fp32)
            nc.vector.tensor_add(denom[:], d1[:], Tdj[2][:])
            r32 = wk.tile([P_OUT, 560], fp32)
            nc.vector.reciprocal_approx_fast(r32[:], denom[:])
            r16 = wk.tile([P_OUT, 560], fp16)
            nc.scalar.copy(r16[:], r32[:])
            out = wk.tile([P_OUT, OF], fp16)

            nj = wk.tile([P_OUT, 560], fp16)
            nc.vector.tensor_sub(nj[:], Tdj[2][:], Tdj[0][:])
            nc.vector.tensor_mul(out[:, 560:1120], nj[:], r16[:])
            nc.scalar.dma_start(od[:, 560:1120], out[:, 560:1120])

            nl = wk.tile([P_OUT, 560], fp16)
            nc.vector.tensor_add(nl[:], nl1[:], dls[2][:])
            nc.vector.tensor_mul(out[:, 1120:1680], nl[:], r16[:])
            nc.sync.dma_start(od[:, 1120:1680], out[:, 1120:1680])

            v2 = wk.tile([P_OUT, 1680], fp16)
            nc.vector.tensor_add(v2[:], v1[:], vs[2][:])
            ni1 = wk.tile([P_OUT, 560], fp16)
            nc.vector.tensor_add(ni1[:], v2[:, 0:560], v2[:, 560:1120])
            ni = wk.tile([P_OUT, 560], fp16)
            nc.vector.tensor_add(ni[:], ni1[:], v2[:, 1120:1680])
            nc.vector.tensor_mul(out[:, 0:560], ni[:], r16[:])
            nc.scalar.dma_start(od[:, 0:560], out[:, 0:560])

    nc.compile()
    return nc


_PERM = np.array([(c % 8) * 6 + c // 8 for c in range(DIM)])


def _host_prep(q, k, rpb):
    f16 = np.float16
    q0 = (np.asarray(q, np.float32)[0] * SCALE).astype(f16)[..., _PERM]
    k0 = np.asarray(k, np.float32)[0].astype(f16)[..., _PERM]
    kp = np.pad(k0, ((1, 1), (1, 1), (1, 3), (0, 0)))     # [42,42,44,48]
    qp = np.pad(q0, ((0, 0), (0, 0), (0, 2), (0, 0)))     # [40,40,42,48]

    rpb16 = (np.asarray(rpb, np.float32) - SHIFT).astype(f16)  # [8,3,3,3]
    rpb_t = np.ascontiguousarray(
        rpb16.transpose(1, 2, 3, 0)).reshape(27 * NH)     # (di,dj,dl,h)
    rpbx = np.broadcast_to(rpb_t, (P_OUT, 27 * NH)).copy()

    in_maps = []
    for core in range(N_CORES):
        i0 = core * SLAB
        qs = qp[i0:i0 + SLAB]
        qt = qs.reshape(SLAB, W, NLC, LCH, DIM).transpose(1, 2, 0, 3, 4)
        qt = np.ascontiguousarray(qt).reshape(P_OUT, QF)
        ks = kp[i0:i0 + KI]
        idx_l = (np.arange(NLC)[:, None] * LCH + np.arange(KL)[None, :])
        kk = ks[:, :, idx_l]
        kk = kk.transpose(1, 2, 0, 3, 4)
        kk = np.ascontiguousarray(kk).reshape(P_K, KF)
        in_maps.append({"qd": qt, "kd": kk, "rd": rpbx})
    return in_maps


def _assemble(results):
    full = np.zeros((H, W, T, NH, 3), np.float32)
    for core in range(N_CORES):
        i0 = core * SLAB
        o = results[core]["od"].astype(np.float32)
        o = o.reshape(W, NLC, 3, SLAB, LCH, NH)
        for lc in range(NLC):
            nl_ = LCH if lc < 2 else T - 2 * LCH
            full[i0:i0 + SLAB, :, lc * LCH:lc * LCH + nl_] = (
                o[:, lc, :, :, :nl_].transpose(2, 0, 3, 4, 1))
    out = full.reshape(H, W, T, NH * 3).transpose(3, 0, 1, 2)[None]
    return np.ascontiguousarray(out)


def _run(q, k, rpb, **spmd_kwargs):
    if "prog" not in _prog_cache:
        _prog_cache["prog"] = _build_program()
    nc = _prog_cache["prog"]
    in_maps = _host_prep(q, k, rpb)
    res = run_bass_kernel_spmd(nc, in_maps, list(range(N_CORES)),
                               **spmd_kwargs)
    return _assemble(res.results), res


def kernel(q, k, rpb):
    out, _ = _run(q, k, rpb)
    return out
